# revision 51
# baseline (speedup 1.0000x reference)
"""Trainium2 Bass kernel for the FermiNet-style single-configuration ansatz.

Computes log|psi| = logdet(orb_u) + logdet(orb_d) for one electron
configuration. The whole forward runs replicated on 8 NeuronCores (the
problem is tiny; inter-core collectives have a ~7-20us latency floor that
dwarfs the ~1 GFLOP of compute, so replication is the fastest correct
distribution) and core 0's scalar output is returned.

Layout choices (see inline comments):
  - p-tensor kept transposed+doubled: pT2[q, j*64 + i_local], q<64 = feature g
    for spin-up electrons (i<64), q>=64 = feature g for spin-down. This makes
    the per-pair feature matmul a K=64 contraction over partitions, lets
    spin-up/down run concurrently in separate PE array quadrants
    (tile_position), and makes the i-mean a free-dim segmented reduce.
  - s-layer matmuls are plain f32: f32r's truncation noise gets
    chaos-amplified through the 4-layer chain and the ill-conditioned logdet
    (measured ~100 absolute shift on HW); 4 cycles/col is the price.
  - logdet via unpivoted rank-1 Gaussian elimination on the stacked [A_u;A_d]
    [128,64] tile. Per step: ONE fused extract+broadcast matmul (precomputed
    one-hot selector SELS_k [128,128] pulls row k / 64+k of A to every
    partition of its half, restricted to live columns k:), then three DVE
    ops (pivot reciprocal; masked multiplier column; fused
    scalar_tensor_tensor rank-1 update A += m2 * UB). ~1.6us/step, fully
    latency-bound: the chain MM -> DVE -> MM is serial, LDWEIGHTS (~500ns,
    M=128 f32) cannot be hoisted (standalone InstLdweights is broken for
    f32), fp16/f32r operands fail precision or the pre-rounding ISA rule,
    and DVE op costs are free-size-bound so u/d-split pipelining loses.
    Unpivoted LU is stable here (growth factor ~700, logdet error ~6e-3 in
    f32 vs 2e-2 rtol).
  - scheduling: rr-path before ra-path (p-stack starts ~15us in), weight
    DMAs 6-deep prefetch, p-mean reduces emitted lazily in two phases inside
    the consuming s-layer, pm chunk last, layer tanh split in halves, the
    l2 residual pre-added on DVE (t12), Ln table pre-warmed during the LU.

Known HW pitfalls hit during development (do not reintroduce):
  - AluOpType.divide and abs_max are rejected by the DVE ISA check.
  - tensor_tensor_reduce and two same-tile column-range PSUM accumulation
    groups hard-fault the exec unit (NRT_EXEC_UNIT_UNRECOVERABLE).
  - mixed-dtype matmul operands are rejected; f32r operands must be
    produced pre-rounded to f32r by their writer.
"""

import numpy as np

import concourse.bass as bass
import concourse.bacc as bacc
import concourse.mybir as mybir
import concourse.tile as tile
from concourse import bass_isa, masks

F32 = mybir.dt.float32
F32R = mybir.dt.float32r
FP16 = mybir.dt.float16
AF = mybir.ActivationFunctionType
Alu = mybir.AluOpType

NE, NA, NSV, NPV, NU = 128, 32, 512, 64, 64

INPUT_SPECS = [
    ("r", (128, 3)), ("a", (32, 3)),
    ("V0_w", (392, 512)), ("V0_b", (512,)),
    ("V1_w", (1664, 512)), ("V1_b", (512,)),
    ("V2_w", (1664, 512)), ("V2_b", (512,)),
    ("W0_w", (4, 64)), ("W0_b", (64,)),
    ("W1_w", (64, 64)), ("W1_b", (64,)),
    ("W2_w", (64, 64)), ("W2_b", (64,)),
    ("after_w", (1664, 512)), ("after_b", (512,)),
    ("vhu_w", (512, 256)), ("vhu_b", (256,)),
    ("vhd_w", (512, 256)), ("vhd_b", (256,)),
    ("wu_w", (256, 64)), ("wu_b", (64,)),
    ("wd_w", (256, 64)), ("wd_b", (64,)),
]


def _program(tc, nc, ins, out_d, dbg_d=None):
    import os
    stage = os.environ.get("KSTAGE", "full")
    ctx_pools = {}

    def pool(name, bufs, space="SBUF"):
        if name not in ctx_pools:
            ctx_pools[name] = tc.alloc_tile_pool(name=name, bufs=bufs,
                                                 space=space)
        return ctx_pools[name]

    const = pool("const", 1)
    work = pool("work", 1)
    pipe2 = pool("pipe2", 4)
    sbcast = pool("sbcast", 8)
    svtp = pool("svtp", 4)
    big = pool("big", 1)
    wstream = pool("wstream", 6)
    lu = pool("lu", 3)
    ps_big = pool("ps_big", 2, space="PSUM")
    ps_sm = pool("ps_sm", 4, space="PSUM")
    ps_s = ps_big

    dma = nc.sync.dma_start

    def release_all():
        for p in reversed(list(ctx_pools.values())):
            p.release()

    # ---------------- constants ----------------
    ident = const.tile([128, 128], F32, tag="ident")
    masks.make_identity(nc, ident[:])
    ones_row = const.tile([1, 128], F32, tag="ones_row")
    nc.gpsimd.memset(ones_row[:], 1.0)

    def secondary_consts():
        """Constants not needed in the first few microseconds -- emitted
        after the geometry-critical GpSimd work so the first matmuls are
        not queued behind these builds."""
        ones128 = const.tile([128, 128], F32, tag="ones128")
        nc.gpsimd.memset(ones128[:], 1.0)
        # LU strict-lower mask, pre-negated: maskNEG[p,k] = -1 iff (p%64) > k
        # (affine_select indexes partitions view-relative, so the same base
        # works for both halves).
        maskNEG = const.tile([128, 64], F32, tag="maskNEG")
        nc.gpsimd.memset(maskNEG[:], -1.0)
        for half in range(2):
            nc.gpsimd.affine_select(
                out=maskNEG[half * 64:(half + 1) * 64, :],
                in_=maskNEG[half * 64:(half + 1) * 64, :],
                pattern=[[-1, 64]], compare_op=Alu.is_ge,
                fill=0.0, base=-1, channel_multiplier=1)
        # UD[q, p] = 1 iff p < 64 (partition-independent half indicator)
        UD = const.tile([128, 128], F32, tag="UD")
        nc.gpsimd.memset(UD[:, 0:64], 1.0)
        nc.gpsimd.memset(UD[:, 64:128], 0.0)
        # SELDIFF[:, k] = ident[:, k] - ident[:, 64+k]
        SELDIFF = const.tile([128, 64], F32, tag="SELDIFF")
        nc.vector.tensor_tensor(SELDIFF[:], ident[:, 0:64],
                                ident[:, 64:128], op=Alu.subtract)
        # diagmask[p, j] = 1 iff j == p % 64 (diagonal gather for logdet)
        diagmask = const.tile([128, 64], F32, tag="diagmask")
        nc.vector.tensor_tensor(diagmask[:], ident[:, 0:64],
                                ident[:, 64:128], op=Alu.add)
        # UDinv[i, 0] = (i < 64)/64, UDinv[i, 1] = (i >= 64)/64
        UDinv = const.tile([128, 2], F32, tag="UDinv")
        nc.gpsimd.memset(UDinv[:], 0.0)
        nc.gpsimd.memset(UDinv[0:64, 0:1], 1.0 / 64.0)
        nc.gpsimd.memset(UDinv[64:128, 1:2], 1.0 / 64.0)
        return ones128, maskNEG, UD, SELDIFF, diagmask, UDinv

    # ---------------- geometry ----------------
    # rr-path first: the p-stack's layer 0 depends only on rr/rr_len, so
    # emitting it (and the transposed pT2 build) before the ra-path lets
    # the p-layers start ~10us earlier. The two sqrt activations are
    # adjacent and exp comes after, so each ACT table loads once.
    # Dummy activations pre-warm the tanh+sqrt tables during the idle boot
    # window (tanh first: if the ACT has a single table slot, the sqrt load
    # below evicts it and we only break even; with two slots both critical
    # loads disappear).
    warm = work.tile([1, 1], F32, tag="warm")
    nc.scalar.activation(warm[:], ones_row[:, 0:1], AF.Tanh)
    nc.scalar.activation(warm[:], ones_row[:, 0:1], AF.Sqrt)
    r_sb = work.tile([128, 3], F32, tag="r_sb")
    dma(r_sb[:], ins["r"][:])
    rrow = const.tile([1, 3 * NE], F32, tag="rrow")
    for c in range(3):
        dma(rrow[:, c * NE:(c + 1) * NE],
            ins["r"][:, c:c + 1].rearrange("a b -> (a b)"))
    # row vectors for the K=1 "subtract a / add r[j]" matmul contributions
    arow = pipe2.tile([1, 3 * NA], F32, tag="arow")
    dma(arow[:], ins["a"][:].rearrange("a b -> (a b)"))
    arow_neg = const.tile([1, 3 * NA], F32, tag="arow_neg")
    nc.vector.tensor_scalar_mul(arow_neg[:], arow[:], -1.0)

    # rT4 = [r^T ; ones] as [4, 128] (row 3 = ones, kept for pm0 lhsT ones)
    psr = ps_sm.tile([4, 128], F32, tag="small")
    nc.tensor.transpose(psr[0:3, :], r_sb[:], ident[:])
    rT4 = const.tile([4, 128], F32, tag="rT4")
    nc.gpsimd.memset(rT4[:], 1.0)  # row 3 stays ones
    nc.vector.tensor_copy(rT4[0:3, :], psr[0:3, :])

    def delta_rows(t, nj, val):
        """t[c, j*3+k] = val*(k == c) for c in 0..2.
        Compute-engine APs must start at partition 0/32/64/96, so build the
        delta pattern with one affine_select over all 4 rows."""
        nc.gpsimd.memset(t[:], val)
        nc.gpsimd.affine_select(
            out=t[:], in_=t[:], pattern=[[0, nj], [1, 3]],
            compare_op=Alu.is_equal, fill=0.0, base=0, channel_multiplier=-1)

    # rr[i, c*128+j] = r[j,c] - r[i,c] in C-MAJOR layout (three contiguous
    # 128-wide blocks): the pT2 transposes then read contiguous slices
    # directly (no strided staging copy) and the squared-length reduce
    # becomes two plain adds.
    Wrr = const.tile([4, 3 * NE], F32, tag="Wrr")
    nc.gpsimd.memset(Wrr[:], -1.0)
    nc.gpsimd.affine_select(
        out=Wrr[:], in_=Wrr[:], pattern=[[1, 3], [0, NE]],
        compare_op=Alu.is_equal, fill=0.0, base=0, channel_multiplier=-1)

    ps_rr = ps_sm.tile([128, 3 * NE], F32, tag="small")
    nc.tensor.matmul(ps_rr[:], rT4[0:3, :], Wrr[0:3, :],
                     start=True, stop=False)
    nc.tensor.matmul(ps_rr[:], ones_row[:], rrow[:],
                     start=False, stop=True)
    rr_sb = work.tile([128, 3 * NE], F32, tag="rr_sb")
    nc.vector.tensor_copy(rr_sb[:], ps_rr[:])
    rr2 = work.tile([128, 3 * NE], F32, tag="rr2")
    nc.scalar.square(rr2[:], ps_rr[:])
    rr_len2 = work.tile([128, NE], F32, tag="rr_len2")
    nc.vector.tensor_tensor(rr_len2[:], rr2[:, 0:NE], rr2[:, NE:2 * NE],
                            op=Alu.add)
    nc.vector.tensor_tensor(rr_len2[:], rr_len2[:], rr2[:, 2 * NE:3 * NE],
                            op=Alu.add)
    rr_len = work.tile([128, NE], F32, tag="rr_len")
    nc.scalar.sqrt(rr_len[:], rr_len2[:])  # diagonal is exactly 0

    # ---------------- pT2_0: p_v0 in transposed-doubled layout ----------------
    # pT2_0[g, j*64+il] = p_v0[il, j, g] (u half, partitions 0..3)
    # pT2_0[64+g, ...] = p_v0[64+il, j, g] (d half, partitions 64..67)
    # The partition->free flattening DMAs are split by j-half so layer 0's
    # first chunks can start as soon as the low-j halves land.
    pT2_0 = big.tile([128, 8192], FP16, tag="pT2_0")
    for g in range(4):
        # c-major rr blocks are contiguous, so the TensorE transpose (which
        # silently no-ops for strided inputs on HW) reads them directly
        src = rr_sb[:, g * 128:(g + 1) * 128] if g < 3 else rr_len[:]
        pst = ps_sm.tile([128, 128], F32, tag="small")
        nc.tensor.transpose(pst[:], src, ident[:])  # pst[j, i] = p0[i, j, g]
        pstc = pipe2.tile([128, 128], FP16, tag="p0T")
        nc.vector.tensor_copy(pstc[:], pst[:])
        du = pT2_0[g:g + 1, :].rearrange("p (j i) -> p j i", i=64)
        dd = pT2_0[64 + g:65 + g, :].rearrange("p (j i) -> p j i", i=64)
        for jh in range(2):
            jsl = slice(jh * 64, (jh + 1) * 64)
            dma(du[:, jsl, :], pstc[jsl, 0:64])
            dma(dd[:, jsl, :], pstc[jsl, 64:128])

    ones128, maskNEG, UD, SELDIFF, diagmask, UDinv = secondary_consts()

    # ---------------- ra-path (feeds s_v0 / e_col / pm0; all consumed
    # by the s-stack, later than the p-stack) ----------------
    Wra = const.tile([4, 3 * NA], F32, tag="Wra")
    delta_rows(Wra, NA, 1.0)

    ps_ra = ps_sm.tile([128, 3 * NA], F32, tag="small")
    nc.tensor.matmul(ps_ra[:], rT4[0:3, :], Wra[0:3, :],
                     start=True, stop=False)
    nc.tensor.matmul(ps_ra[:], ones_row[:], arow_neg[:],
                     start=False, stop=True)
    ra_sb = work.tile([128, 3 * NA], F32, tag="ra_sb")
    nc.vector.tensor_copy(ra_sb[:], ps_ra[:])
    ra2 = work.tile([128, 3 * NA], F32, tag="ra2")
    nc.scalar.square(ra2[:], ps_ra[:])
    ra_len2 = work.tile([128, NA], F32, tag="ra_len2")
    nc.vector.reduce_sum(
        ra_len2[:], ra2[:].rearrange("p (j c) -> p j c", c=3),
        axis=mybir.AxisListType.X,
    )
    ra_len = work.tile([128, NA], F32, tag="ra_len")
    nc.scalar.sqrt(ra_len[:], ra_len2[:])
    # e_col[i] = sum_j exp(-|r_i - a_j|)
    e_col = const.tile([128, 1], F32, tag="e_col")
    eexp = work.tile([128, NA], F32, tag="eexp")
    nc.scalar.activation(eexp[:], ra_len[:], AF.Exp, scale=-1.0,
                         accum_out=e_col[:])

    def dbg_out(src_ap):
        o = work.tile([1, 1], F32, tag="out_sb")
        nc.scalar.mul(o[:], src_ap, 1.0)
        dma(out_d[:], o[:])

    # s_v0 [128, 128]: interleaved [ra_x, ra_y, ra_z, |ra|] per atom
    s_v0 = work.tile([128, 128], F32, tag="s_v0")
    v4 = s_v0[:].rearrange("p (j k) -> p j k", k=4)
    nc.vector.tensor_copy(v4[:, :, 0:3],
                          ra_sb[:].rearrange("p (j c) -> p j c", c=3))
    nc.vector.tensor_copy(v4[:, :, 3:4],
                          ra_len[:].rearrange("p (j k) -> p j k", k=1))

    if stage == "geom":
        dbg_out(e_col[0:1, :])
        release_all()
        return

    # ---------------- pm0: layer-0 p-means, built analytically --------------
    # mean over an i-half of rr[i, j, c] = r[j, c] - rbar_half[c] for the
    # three coordinate features (the i=j diagonal term is 0 on both sides);
    # only the |rr| feature needs a real mean (one matmul against rr_len,
    # transposed to rows via the PE). pm0 rows: 0-3 = u [x y z len],
    # 4-7 = d, row 8 = ones (carries the V0 bias as a 9th contraction row).
    # Rows 3..8 live at compute-unaddressable partitions, so they are filled
    # by SBUF-to-SBUF DMA, all off the critical path.
    pm0 = work.tile([128, 128], F32, tag="pm0")
    dma(pm0[8:9, :], ones_row[:])
    ps_rb = ps_sm.tile([3, 2], F32, tag="small")
    nc.tensor.matmul(ps_rb[:], r_sb[:], UDinv[:], start=True, stop=True)
    nc.vector.tensor_scalar_sub(pm0[0:3, :], rT4[0:3, :], ps_rb[0:3, 0:1])
    pm0d = pipe2.tile([3, 128], F32, tag="pm0d")
    nc.vector.tensor_scalar_sub(pm0d[:], rT4[0:3, :], ps_rb[0:3, 1:2])
    dma(pm0[4:7, :], pm0d[:])
    ps_L = ps_sm.tile([128, 2], F32, tag="small")
    nc.tensor.matmul(ps_L[:], rr_len[:], UDinv[:], start=True, stop=True)
    lcol = pipe2.tile([128, 2], F32, tag="lcol")
    nc.vector.tensor_copy(lcol[:], ps_L[:])
    ps_LT = ps_sm.tile([2, 128], F32, tag="small")
    nc.tensor.transpose(ps_LT[:], lcol[:], ident[:])
    lrow = pipe2.tile([2, 128], F32, tag="lrow")
    nc.vector.tensor_copy(lrow[:], ps_LT[:])
    dma(pm0[3:4, :], lrow[0:1, :])
    dma(pm0[7:8, :], lrow[1:2, :])

    # ---------------- p-layer weights ----------------
    # Layer 0 (K=4): weights doubled to both partition halves, used by two
    # quadrant (tile_position) matmuls. Layers 1/2 (K=64): BLOCK-DIAGONAL
    # [128,128] weights [[W,0],[0,W]] -- since the t-tiles hold u-features
    # on partitions 0:64 and d-features on 64:128, ONE full-K matmul then
    # computes both halves (halves the p-layer matmul count; the zero
    # blocks make the math identical).
    Wp, Wpb, Kp = [], [], [4, 64, 64]
    for l, (wn, bn) in enumerate([("W0_w", "W0_b"), ("W1_w", "W1_b"),
                                  ("W2_w", "W2_b")]):
        K = Kp[l]
        wstage = pipe2.tile([64, 64], F32, tag="wstage")
        dma(wstage[0:K, :], ins[wn][:])
        if l == 0:
            wt = const.tile([128, 64], FP16, tag=f"wp{l}")
            nc.vector.tensor_copy(wt[0:K, :], wstage[0:K, :])
            nc.vector.tensor_copy(wt[64:64 + K, :], wstage[0:K, :])
        else:
            wt = const.tile([128, 128], FP16, tag=f"wp{l}")
            nc.gpsimd.memset(wt[:], 0.0)
            nc.vector.tensor_copy(wt[0:64, 0:64], wstage[:])
            nc.vector.tensor_copy(wt[64:128, 64:128], wstage[:])
        bc = const.tile([128, 1], F32, tag=f"wpb{l}")
        dma(bc[0:64, :], ins[bn][:].rearrange("(a k) -> a k", k=1))
        dma(bc[64:128, :], ins[bn][:].rearrange("(a k) -> a k", k=1))
        Wp.append(wt)
        Wpb.append(bc)

    # ---------------- LU selector matrices (built early on idle GpSimd) ----
    # SELS_k = SELS[:, k*128:(k+1)*128], SELS_k[q, p] = 1 iff
    # (q == k and p < 64) or (q == 64+k and p >= 64): the lhsT of a single
    # matmul that extracts pivot rows k (u) / 64+k (d) of A and broadcasts
    # each to its 64-partition half. SELS_k = UD * SELDIFF[:,k] + ident[:,64+k]
    SELS = const.tile([128, 64 * 128], F32, tag="SELS")
    for k in range(64):
        nc.gpsimd.tensor_scalar(
            SELS[:, k * 128:(k + 1) * 128], UD[:],
            SELDIFF[:, k:k + 1], ident[:, 64 + k:65 + k],
            op0=Alu.mult, op1=Alu.add)

    # ---------------- p-layers ----------------
    # t_{l+1} = tanh(W_l^T applied to p_v_l); p_v residuals kept distributed.
    t_tiles = []

    def p_layer(l, rhs_list, out_tag=None):
        """rhs_list: list of (tile, K) contributions summed pre-tanh."""
        out_t = big.tile([128, 8192], FP16, tag=out_tag or f"t{l + 1}")
        wt, bc = Wp[l], Wpb[l]
        for c in range(16):
            ps = ps_big.tile([128, 512], F32, tag="big512")
            sl = slice(c * 512, (c + 1) * 512)
            n = len(rhs_list)
            for idx, (src, K) in enumerate(rhs_list):
                st, sp = idx == 0, idx == n - 1
                if K == 64:
                    # block-diagonal weights: one K=128 matmul does both
                    # spin halves
                    nc.tensor.matmul(ps[:], wt[:], src[:, sl],
                                     start=st, stop=sp)
                else:
                    # layer 0: independent accumulation group per psum
                    # partition-range via PE quadrants. skip_group_check:
                    # the sim's zero-region tracking is bank-global, but
                    # disjoint-partition groups are sound (per-element
                    # has_written bits); verified numerically.
                    nc.tensor.matmul(ps[0:64, :], wt[0:K, :], src[0:K, sl],
                                     start=st, stop=sp, tile_position=(0, 0))
                    nc.tensor.matmul(ps[64:128, :], wt[64:64 + K, :],
                                     src[64:64 + K, sl],
                                     start=st, stop=sp,
                                     tile_position=(64, 64),
                                     skip_group_check=True)
            nc.scalar.activation(out_t[:, sl], ps[:], AF.Tanh, bias=bc[:])
        t_tiles.append(out_t)
        return out_t

    t1 = p_layer(0, [(pT2_0, 4)])
    t2 = p_layer(1, [(t1, 64)])
    # residual input p_v2 = t1 + t2 pre-added on DVE (chunked so the adds
    # pipeline behind layer 1's tanh) -- halves layer 2's matmul count.
    t12 = big.tile([128, 8192], FP16, tag="t12")
    for c in range(8):
        sl = slice(c * 1024, (c + 1) * 1024)
        nc.vector.tensor_tensor(t12[:, sl], t1[:, sl], t2[:, sl],
                                op=Alu.add)
    # t3 reuses pT2_0's SBUF slot (pT2_0 is dead after layer 0)
    t3 = p_layer(2, [(t12, 64)], out_tag="pT2_0")

    # ---------------- p means (cumulative, scaled 1/64) ----------------
    # red_l[q, j] = sum_il t_l[q, j*64+il]; pmean chunks feed s-matmul lhsT.
    # Cumulative means are pre-summed on DVE (pm12, pm123) so each s-layer
    # needs only ONE pm chunk matmul. The reduces are EMITTED LAZILY (from
    # pm_fn inside each s-layer, with the pm chunk ordered last) so the
    # 8.6us DVE reduce of layer l runs while the PE chews layer l+1's
    # su/sd/sv chunks instead of blocking the layer's broadcast builds.
    pm_state = {}

    def make_pm_fn(t_src, tag, prev_key, out_key):
        """Two-phase lazy i-segment mean of a t-tile: fold_fn emits the
        half-width add (i 0:32 + 32:64, f32), finish_fn the 32-wide
        segmented reduce and the cumulative 1/64-scaled accumulate. The
        phases are emitted at different points inside the consuming s-layer
        so the 6us of DVE reduce work hides behind its chunk matmuls. The
        f32 fold tile is shared across layers (reduces are serial on DVE)."""
        state = {}

        def fold_fn():
            v = t_src[:].rearrange("p (j i) -> p j i", i=64)
            fold = work.tile([128, 4096], F32, tag="redfold")
            nc.vector.tensor_tensor(
                fold[:].rearrange("p (j i) -> p j i", i=32),
                v[:, :, 0:32], v[:, :, 32:64], op=Alu.add)
            state["fold"] = fold

        def finish_fn():
            red = work.tile([128, 128], F32, tag=tag)
            nc.vector.reduce_sum(
                red[:], state["fold"][:].rearrange("p (j i) -> p j i", i=32),
                axis=mybir.AxisListType.X,
            )
            pm = work.tile([128, 128], F32, tag="pm" + tag)
            if prev_key is None:
                nc.vector.tensor_scalar_mul(pm[:], red[:], 1.0 / 64.0)
            else:
                nc.vector.scalar_tensor_tensor(pm[:], red[:], 1.0 / 64.0,
                                               pm_state[prev_key][:],
                                               op0=Alu.mult, op1=Alu.add)
            pm_state[out_key] = pm
            return pm

        return fold_fn, finish_fn

    if stage == "p":
        f, fin = make_pm_fn(t1, "red1", None, "pm1")
        f()
        dbg_out(fin()[0:1, 0:1])
        release_all()
        return

    # ---------------- s-layers ----------------
    def s_means_bcast(s_v, width):
        """Column-mean of the u/d row-halves of s_v, broadcast to [128,128]
        lhsT tiles (bt[k, m] = mean[k]; the matmul replicates the mean-row
        to all 128 output rows). One K=128 matmul against UDinv yields both
        halves' means at once. Returns (su_tiles, sd_tiles) per chunk."""
        nch = width // 128
        su, sd = [], []
        for c in range(nch):
            sl = slice(c * 128, (c + 1) * 128)
            psm = ps_sm.tile([128, 2], F32, tag="small")
            nc.tensor.matmul(psm[:], s_v[:, sl], UDinv[:],
                             start=True, stop=True)
            for half, out_list in ((0, su), (1, sd)):
                bt = sbcast.tile([128, 128], F32, tag="sbcast")
                nc.vector.tensor_scalar_mul(bt[:], ones128[:],
                                            psm[:, half:half + 1])
                out_list.append(bt)
        return su, sd

    def s_transposes(s_v, width):
        out = []
        for c in range(width // 128):
            sl = slice(c * 128, (c + 1) * 128)
            pst = ps_sm.tile([128, 128], F32, tag="small")
            nc.tensor.transpose(pst[:], s_v[:, sl], ident[:])
            svt = svtp.tile([128, 128], F32, tag="svT")
            nc.vector.tensor_copy(svt[:], pst[:])
            out.append(svt)
        return out

    def s_layer(lname, wkey, bkey, chunks, bias_chunk=None):
        """chunks: (lhsT_ap, vw_row_start, K). All matmuls are plain f32
        (see module docstring). bias_chunk: index of a chunk whose LAST
        contraction row is the bias (lhsT row = ones); otherwise the bias
        is a trailing rank-1 ones x b matmul. Returns s_v [128,512] f32."""
        ps = ps_s.tile([128, 512], F32, tag="big512")
        n = len(chunks)
        for idx, (lhsT, row0, K) in enumerate(chunks):
            wv = wstream.tile([128, 512], F32, tag="vw")
            if idx == bias_chunk:
                dma(wv[0:K - 1, :], ins[wkey][row0:row0 + K - 1, :])
                dma(wv[K - 1:K, :],
                    ins[bkey][:].rearrange("(k a) -> k a", k=1))
            else:
                dma(wv[0:K, :], ins[wkey][row0:row0 + K, :])
            stop = (idx == n - 1) and bias_chunk is not None
            nc.tensor.matmul(ps[:], lhsT, wv[0:K, :],
                             start=(idx == 0), stop=stop)
        if bias_chunk is None:
            vb = wstream.tile([1, 512], F32, tag="vb")
            dma(vb[:], ins[bkey][:].rearrange("(k a) -> k a", k=1))
            nc.tensor.matmul(ps[:], ones_row[:], vb[:],
                             start=False, stop=True)
        # tanh in two half-width pieces: the next layer's mean matmuls and
        # transposes for the low columns start ~300ns earlier, shrinking the
        # serial layer-boundary latency.
        s_v = work.tile([128, 512], F32, tag=f"sv{lname}")
        nc.scalar.activation(s_v[:, 0:256], ps[:, 0:256], AF.Tanh)
        nc.scalar.activation(s_v[:, 256:512], ps[:, 256:512], AF.Tanh)
        return s_v

    # layer 0: fin = 392 = su(128) sd(128) pu(4) pd(4) sv(128); bias rides
    # as the 9th row of the pm chunk (pm0 row 8 = ones). pm chunk last.
    su0, sd0 = s_means_bcast(s_v0, 128)
    sv0T = s_transposes(s_v0, 128)
    s_v1 = s_layer(
        "1", "V0_w", "V0_b",
        [(su0[0][:], 0, 128), (sd0[0][:], 128, 128),
         (sv0T[0][:], 264, 128),
         (pm0[0:9, :], 256, 9)],
        bias_chunk=3,
    )

    # layers 1, 2, after: fin = 1664 = su(512) sd(512) pu(64) pd(64) sv(512)
    def big_s_layer(lname, wkey, bkey, s_v, pm_fns):
        fold_fn, finish_fn = pm_fns
        su, sd = s_means_bcast(s_v, 512)
        fold_fn()
        svT = s_transposes(s_v, 512)
        chunks = []
        for c in range(4):
            chunks.append((su[c][:], c * 128, 128))
        for c in range(4):
            chunks.append((sd[c][:], 512 + c * 128, 128))
        for c in range(4):
            chunks.append((svT[c][:], 1152 + c * 128, 128))
        # pu rows 1024:1088 and pd rows 1088:1152 are contiguous in Vw, and
        # pm holds pu-features at partitions 0:64, pd at 64:128 -- one
        # full-array K=128 chunk covers both. Emitted last: the DVE reduce
        # it depends on overlaps the 12 chunk matmuls above.
        pm = finish_fn()
        chunks.append((pm[:], 1024, 128))
        return s_layer(lname, wkey, bkey, chunks)

    if stage == "s1" and dbg_d is not None:
        sv1f = work.tile([128, 512], F32, tag="sv1f")
        nc.scalar.activation(sv1f[:], s_v1[:], AF.Identity)
        dma(dbg_d[:], sv1f[:])
        dbg_out(s_v1[0:1, 0:1])
        release_all()
        return

    s_v2 = big_s_layer("2", "V1_w", "V1_b", s_v1,
                       make_pm_fn(t1, "red1", None, "pm1"))
    s_v3 = big_s_layer("3", "V2_w", "V2_b", s_v2,
                       make_pm_fn(t2, "red2", "pm1", "pm12"))
    s_v4 = big_s_layer("4", "after_w", "after_b", s_v3,
                       make_pm_fn(t3, "red3", "pm12", "pm123"))

    if stage == "s":
        dbg_out(s_v4[0:1, 0:1])
        release_all()
        return

    # ---------------- heads ----------------
    sv4T = s_transposes(s_v4, 512)

    def head_half(wkey, bkey):
        ps = ps_sm.tile([64, 256], F32, tag="small")
        base = 0 if wkey == "vhu_w" else 64
        for c in range(4):
            wv = wstream.tile([128, 256], F32, tag="vhw")
            dma(wv[:], ins[wkey][c * 128:(c + 1) * 128, :])
            nc.tensor.matmul(ps[:], sv4T[c][:, base:base + 64],
                             wv[:], start=(c == 0), stop=False)
        vb = wstream.tile([1, 256], F32, tag="vhb")
        dma(vb[:], ins[bkey][:].rearrange("(k a) -> k a", k=1))
        nc.tensor.matmul(ps[:], ones_row[:, 0:64], vb[:],
                         start=False, stop=True)
        sh = work.tile([64, 256], F32, tag="sh" + wkey)
        nc.vector.tensor_copy(sh[:], ps[:])
        return sh

    shu = head_half("vhu_w", "vhu_b")
    shd = head_half("vhd_w", "vhd_b")

    def head_T(sh):
        out = []
        for c in range(2):
            pst = ps_sm.tile([128, 128], F32, tag="small")
            nc.tensor.transpose(pst[0:128, 0:64],
                                sh[:, c * 128:(c + 1) * 128],
                                ident[0:64, 0:64])
            ht = svtp.tile([128, 64], F32, tag="ht")
            nc.vector.tensor_copy(ht[:], pst[0:128, 0:64])
            out.append(ht)
        return out

    shuT = head_T(shu)
    shdT = head_T(shd)

    ps_A = ps_sm.tile([128, 64], F32, tag="small")
    for c in range(2):
        wv = wstream.tile([128, 64], F32, tag="ww")
        dma(wv[:], ins["wu_w"][c * 128:(c + 1) * 128, :])
        nc.tensor.matmul(ps_A[0:64, :], shuT[c][:], wv[:],
                         start=(c == 0), stop=False, tile_position=(0, 0))
    vbu = wstream.tile([1, 64], F32, tag="wb")
    dma(vbu[:], ins["wu_b"][:].rearrange("(k a) -> k a", k=1))
    nc.tensor.matmul(ps_A[0:64, :], ones_row[:, 0:64], vbu[:],
                     start=False, stop=True, tile_position=(0, 0))
    for c in range(2):
        wv = wstream.tile([128, 64], F32, tag="ww")
        dma(wv[:], ins["wd_w"][c * 128:(c + 1) * 128, :])
        nc.tensor.matmul(ps_A[64:128, :], shdT[c][:], wv[:],
                         start=(c == 0), stop=False, tile_position=(0, 64))
    vbd = wstream.tile([1, 64], F32, tag="wb")
    dma(vbd[:], ins["wd_b"][:].rearrange("(k a) -> k a", k=1))
    nc.tensor.matmul(ps_A[64:128, :], ones_row[:, 0:64], vbd[:],
                     start=False, stop=True, tile_position=(0, 64))

    # orb = s_w * (sum_j exp(-|ra|)) row-scale; stacked [A_u; A_d]
    A_sb = work.tile([128, 64], F32, tag="A_sb")
    nc.vector.tensor_scalar_mul(A_sb[:], ps_A[:], e_col[:])
    # dummy Ln: pulls the ACT table load for the logdet epilogue off the
    # end of the serial LU (the ACT engine is idle for the whole LU)
    lnwarm = work.tile([1, 1], F32, tag="lnwarm")
    nc.scalar.activation(lnwarm[:], ones_row[:, 0:1], AF.Ln)

    if stage == "heads":
        dbg_out(A_sb[0:1, 0:1])
        release_all()
        return

    # ---------------- stacked unpivoted LU ----------------
    # Per step k: one matmul (lhsT = SELS_k) extracts pivot rows k / 64+k of
    # A and broadcasts each across its partition half into PSUM; then 3 DVE
    # ops: R = 1/UB[:,k] (the DVE has no divide ALU op -- the ISA check
    # rejects it), m2 = A[:,k] * R * maskNEG[:,k] (zero for rows <= k, so
    # finished U rows and the diagonal are never touched) and the fused
    # rank-1 update A = UB * m2 + A over the full 64-col width (dead columns
    # only ever receive ~eps noise; the multiplier column k is eliminated to
    # ~0, which is dead anyway). Step 63 does nothing: row 63's diagonal was
    # finalized by step 62. The U diagonal is gathered from A at the end.
    n_lu = {"lu16": 16, "lu64": 63}.get(stage, 63)
    for k in range(n_lu):
        # only the active columns k..63 are extracted and updated; stale
        # sub-diagonal columns are dead (every live (row k', col >= k')
        # entry has received all updates < k' because step t covers cols
        # t+1..63 in full).
        w = 64 - k
        ub = ps_sm.tile([128, w], F32, tag="small")
        nc.tensor.matmul(ub[:], SELS[:, k * 128:(k + 1) * 128],
                         A_sb[:, k:64], start=True, stop=True)
        rc = lu.tile([128, 1], F32, tag="rc")
        nc.vector.reciprocal(rc[:], ub[:, 0:1])
        m2 = lu.tile([128, 1], F32, tag="m2")
        nc.vector.tensor_scalar(m2[:], A_sb[:, k:k + 1], rc[:],
                                maskNEG[:, k:k + 1],
                                op0=Alu.mult, op1=Alu.mult)
        nc.vector.scalar_tensor_tensor(A_sb[:, k + 1:64], ub[:, 1:w],
                                       m2[:], A_sb[:, k + 1:64],
                                       op0=Alu.mult, op1=Alu.add)

    if stage in ("lu16", "lu64"):
        dbg_out(A_sb[0:1, 0:1])
        release_all()
        return

    # logdet = allreduce_p( ln|A[p, p%64]| )  (each partition holds one
    # diagonal element of its half's U factor)
    scr = work.tile([128, 64], F32, tag="scr")
    diag_col = work.tile([128, 1], F32, tag="diag_col")
    nc.vector.tensor_tensor(scr[:], A_sb[:], diagmask[:], op=Alu.mult)
    nc.vector.reduce_sum(diag_col[:], scr[:], axis=mybir.AxisListType.X)
    # ln|d| = ln(d^2)/2 -- avoids the Abs table load (only Ln loads)
    sq_col = work.tile([128, 1], F32, tag="sq_col")
    nc.vector.tensor_tensor(sq_col[:], diag_col[:], diag_col[:],
                            op=Alu.mult)
    ln_col = work.tile([128, 1], F32, tag="ln_col")
    nc.scalar.activation(ln_col[:], sq_col[:], AF.Ln)
    # cross-partition sum: transpose the column to a row, reduce along free
    ps_out = ps_sm.tile([1, 128], F32, tag="small")
    nc.tensor.transpose(ps_out[:], ln_col[:], ident[:])
    s_row = work.tile([1, 128], F32, tag="s_row")
    nc.vector.tensor_scalar_mul(s_row[:], ps_out[:], 0.5)
    out_sb = work.tile([1, 1], F32, tag="out_sb")
    nc.vector.reduce_sum(out_sb[:], s_row[:], axis=mybir.AxisListType.X)
    dma(out_d[:], out_sb[:])

    release_all()


_NC_CACHE = {}


def build_nc():
    if "nc" in _NC_CACHE:
        return _NC_CACHE["nc"]
    import os
    nc = bacc.Bacc("TRN2", target_bir_lowering=False, debug=False)
    ins = {}
    for name, shape in INPUT_SPECS:
        ins[name] = nc.dram_tensor(name, list(shape), F32,
                                   kind="ExternalInput").ap()
    out_d = nc.dram_tensor("out", [1, 1], F32, kind="ExternalOutput").ap()
    dbg_d = None
    if os.environ.get("KSTAGE", "full") in ("s1", "pm0"):
        dbg_d = nc.dram_tensor("dbgout", [128, 512], F32,
                               kind="ExternalOutput").ap()
    with tile.TileContext(nc) as tc:
        _program(tc, nc, ins, out_d, dbg_d)
    nc.compile()
    _NC_CACHE["nc"] = nc
    return nc


def kernel(**inputs) -> np.ndarray:
    from concourse.bass_utils import run_bass_kernel_spmd

    nc = build_nc()
    in_map = {name: np.ascontiguousarray(np.asarray(inputs[name],
                                                    dtype=np.float32))
              for name, _ in INPUT_SPECS}
    in_maps = [in_map for _ in range(8)]
    res = run_bass_kernel_spmd(nc, in_maps, core_ids=list(range(8)))
    out = res.results[0]["out"]
    return np.float32(out.reshape(())[()])


# revision 52
# speedup vs baseline: 1.1747x; 1.1747x over previous
"""Trainium2 Bass kernel for the FermiNet-style single-configuration ansatz.

Computes log|psi| = logdet(orb_u) + logdet(orb_d) for one electron
configuration. The whole forward runs replicated on 8 NeuronCores (the
problem is tiny; inter-core collectives have a ~7-20us latency floor that
dwarfs the ~1 GFLOP of compute, so replication is the fastest correct
distribution) and core 0's scalar output is returned.

Layout choices (see inline comments):
  - p-tensor kept transposed+doubled: pT2[q, j*64 + i_local], q<64 = feature g
    for spin-up electrons (i<64), q>=64 = feature g for spin-down. This makes
    the per-pair feature matmul a K=64 contraction over partitions, lets
    spin-up/down run concurrently in separate PE array quadrants
    (tile_position), and makes the i-mean a free-dim segmented reduce.
  - s-layer matmuls are plain f32: f32r's truncation noise gets
    chaos-amplified through the 4-layer chain and the ill-conditioned logdet
    (measured ~100 absolute shift on HW); 4 cycles/col is the price.
  - logdet via unpivoted rank-1 Gaussian elimination on the stacked [A_u;A_d]
    [128,64] tile. Per step: ONE fused extract+broadcast matmul (precomputed
    one-hot selector SELS_k [128,128] pulls row k / 64+k of A to every
    partition of its half, restricted to live columns k:), then three DVE
    ops (pivot reciprocal; masked multiplier column; fused
    scalar_tensor_tensor rank-1 update A += m2 * UB). ~1.6us/step, fully
    latency-bound: the chain MM -> DVE -> MM is serial, LDWEIGHTS (~500ns,
    M=128 f32) cannot be hoisted (standalone InstLdweights is broken for
    f32), fp16/f32r operands fail precision or the pre-rounding ISA rule,
    and DVE op costs are free-size-bound so u/d-split pipelining loses.
    Unpivoted LU is stable here (growth factor ~700, logdet error ~6e-3 in
    f32 vs 2e-2 rtol).
  - scheduling: rr-path before ra-path (p-stack starts ~15us in), weight
    DMAs 6-deep prefetch, p-mean reduces emitted lazily in two phases inside
    the consuming s-layer, pm chunk last, layer tanh split in halves, the
    l2 residual pre-added on DVE (t12), Ln table pre-warmed during the LU.

Known HW pitfalls hit during development (do not reintroduce):
  - AluOpType.divide and abs_max are rejected by the DVE ISA check.
  - tensor_tensor_reduce and two same-tile column-range PSUM accumulation
    groups hard-fault the exec unit (NRT_EXEC_UNIT_UNRECOVERABLE).
  - mixed-dtype matmul operands are rejected; f32r operands must be
    produced pre-rounded to f32r by their writer.
"""

import numpy as np

import concourse.bass as bass
import concourse.bacc as bacc
import concourse.mybir as mybir
import concourse.tile as tile
from concourse import bass_isa, masks

F32 = mybir.dt.float32
F32R = mybir.dt.float32r
FP16 = mybir.dt.float16
AF = mybir.ActivationFunctionType
Alu = mybir.AluOpType

NE, NA, NSV, NPV, NU = 128, 32, 512, 64, 64

INPUT_SPECS = [
    ("r", (128, 3)), ("a", (32, 3)),
    ("V0_w", (392, 512)), ("V0_b", (512,)),
    ("V1_w", (1664, 512)), ("V1_b", (512,)),
    ("V2_w", (1664, 512)), ("V2_b", (512,)),
    ("W0_w", (4, 64)), ("W0_b", (64,)),
    ("W1_w", (64, 64)), ("W1_b", (64,)),
    ("W2_w", (64, 64)), ("W2_b", (64,)),
    ("after_w", (1664, 512)), ("after_b", (512,)),
    ("vhu_w", (512, 256)), ("vhu_b", (256,)),
    ("vhd_w", (512, 256)), ("vhd_b", (256,)),
    ("wu_w", (256, 64)), ("wu_b", (64,)),
    ("wd_w", (256, 64)), ("wd_b", (64,)),
]


def _program(tc, nc, ins, out_d, dbg_d=None):
    import os
    stage = os.environ.get("KSTAGE", "full")
    ctx_pools = {}

    def pool(name, bufs, space="SBUF"):
        if name not in ctx_pools:
            ctx_pools[name] = tc.alloc_tile_pool(name=name, bufs=bufs,
                                                 space=space)
        return ctx_pools[name]

    const = pool("const", 1)
    work = pool("work", 1)
    pipe2 = pool("pipe2", 4)
    sbcast = pool("sbcast", 8)
    svtp = pool("svtp", 4)
    big = pool("big", 1)
    wstream = pool("wstream", 6)
    lu = pool("lu", 3)
    ps_big = pool("ps_big", 2, space="PSUM")
    ps_sm = pool("ps_sm", 4, space="PSUM")
    ps_s = ps_big

    dma = nc.sync.dma_start

    def release_all():
        for p in reversed(list(ctx_pools.values())):
            p.release()

    # ---------------- constants ----------------
    ident = const.tile([128, 128], F32, tag="ident")
    masks.make_identity(nc, ident[:])
    ones_row = const.tile([1, 128], F32, tag="ones_row")
    nc.gpsimd.memset(ones_row[:], 1.0)

    def secondary_consts():
        """Constants not needed in the first few microseconds -- emitted
        after the geometry-critical GpSimd work so the first matmuls are
        not queued behind these builds."""
        ones128 = const.tile([128, 128], F32, tag="ones128")
        nc.gpsimd.memset(ones128[:], 1.0)
        # LU strict-lower mask, pre-negated: maskNEG[p,k] = -1 iff (p%64) > k
        # (affine_select indexes partitions view-relative, so the same base
        # works for both halves).
        maskNEG = const.tile([128, 64], F32, tag="maskNEG")
        nc.gpsimd.memset(maskNEG[:], -1.0)
        for half in range(2):
            nc.gpsimd.affine_select(
                out=maskNEG[half * 64:(half + 1) * 64, :],
                in_=maskNEG[half * 64:(half + 1) * 64, :],
                pattern=[[-1, 64]], compare_op=Alu.is_ge,
                fill=0.0, base=-1, channel_multiplier=1)
        # UD[q, p] = 1 iff p < 64 (partition-independent half indicator)
        UD = const.tile([128, 128], F32, tag="UD")
        nc.gpsimd.memset(UD[:, 0:64], 1.0)
        nc.gpsimd.memset(UD[:, 64:128], 0.0)
        # SELDIFF[:, k] = ident[:, k] - ident[:, 64+k]
        SELDIFF = const.tile([128, 64], F32, tag="SELDIFF")
        nc.vector.tensor_tensor(SELDIFF[:], ident[:, 0:64],
                                ident[:, 64:128], op=Alu.subtract)
        # diagmask[p, j] = 1 iff j == p % 64 (diagonal gather for logdet)
        diagmask = const.tile([128, 64], F32, tag="diagmask")
        nc.vector.tensor_tensor(diagmask[:], ident[:, 0:64],
                                ident[:, 64:128], op=Alu.add)
        # UDinv[i, 0] = (i < 64)/64, UDinv[i, 1] = (i >= 64)/64
        UDinv = const.tile([128, 2], F32, tag="UDinv")
        nc.gpsimd.memset(UDinv[:], 0.0)
        nc.gpsimd.memset(UDinv[0:64, 0:1], 1.0 / 64.0)
        nc.gpsimd.memset(UDinv[64:128, 1:2], 1.0 / 64.0)
        return ones128, maskNEG, UD, SELDIFF, diagmask, UDinv

    # ---------------- geometry ----------------
    # rr-path first: the p-stack's layer 0 depends only on rr/rr_len, so
    # emitting it (and the transposed pT2 build) before the ra-path lets
    # the p-layers start ~10us earlier. The two sqrt activations are
    # adjacent and exp comes after, so each ACT table loads once.
    # Dummy activations pre-warm the tanh+sqrt tables during the idle boot
    # window (tanh first: if the ACT has a single table slot, the sqrt load
    # below evicts it and we only break even; with two slots both critical
    # loads disappear).
    warm = work.tile([1, 1], F32, tag="warm")
    nc.scalar.activation(warm[:], ones_row[:, 0:1], AF.Tanh)
    nc.scalar.activation(warm[:], ones_row[:, 0:1], AF.Sqrt)
    r_sb = work.tile([128, 3], F32, tag="r_sb")
    dma(r_sb[:], ins["r"][:])
    rrow = const.tile([1, 3 * NE], F32, tag="rrow")
    for c in range(3):
        dma(rrow[:, c * NE:(c + 1) * NE],
            ins["r"][:, c:c + 1].rearrange("a b -> (a b)"))
    # row vectors for the K=1 "subtract a / add r[j]" matmul contributions
    arow = pipe2.tile([1, 3 * NA], F32, tag="arow")
    dma(arow[:], ins["a"][:].rearrange("a b -> (a b)"))
    arow_neg = const.tile([1, 3 * NA], F32, tag="arow_neg")
    nc.vector.tensor_scalar_mul(arow_neg[:], arow[:], -1.0)

    # rT4 = [r^T ; ones] as [4, 128] (row 3 = ones, kept for pm0 lhsT ones)
    psr = ps_sm.tile([4, 128], F32, tag="small")
    nc.tensor.transpose(psr[0:3, :], r_sb[:], ident[:])
    rT4 = const.tile([4, 128], F32, tag="rT4")
    nc.gpsimd.memset(rT4[:], 1.0)  # row 3 stays ones
    nc.vector.tensor_copy(rT4[0:3, :], psr[0:3, :])

    def delta_rows(t, nj, val):
        """t[c, j*3+k] = val*(k == c) for c in 0..2.
        Compute-engine APs must start at partition 0/32/64/96, so build the
        delta pattern with one affine_select over all 4 rows."""
        nc.gpsimd.memset(t[:], val)
        nc.gpsimd.affine_select(
            out=t[:], in_=t[:], pattern=[[0, nj], [1, 3]],
            compare_op=Alu.is_equal, fill=0.0, base=0, channel_multiplier=-1)

    # rr[i, c*128+j] = r[j,c] - r[i,c] in C-MAJOR layout (three contiguous
    # 128-wide blocks): the pT2 transposes then read contiguous slices
    # directly (no strided staging copy) and the squared-length reduce
    # becomes two plain adds.
    Wrr = const.tile([4, 3 * NE], F32, tag="Wrr")
    nc.gpsimd.memset(Wrr[:], -1.0)
    nc.gpsimd.affine_select(
        out=Wrr[:], in_=Wrr[:], pattern=[[1, 3], [0, NE]],
        compare_op=Alu.is_equal, fill=0.0, base=0, channel_multiplier=-1)

    ps_rr = ps_sm.tile([128, 3 * NE], F32, tag="small")
    nc.tensor.matmul(ps_rr[:], rT4[0:3, :], Wrr[0:3, :],
                     start=True, stop=False)
    nc.tensor.matmul(ps_rr[:], ones_row[:], rrow[:],
                     start=False, stop=True)
    rr_sb = work.tile([128, 3 * NE], F32, tag="rr_sb")
    nc.vector.tensor_copy(rr_sb[:], ps_rr[:])
    rr2 = work.tile([128, 3 * NE], F32, tag="rr2")
    nc.scalar.square(rr2[:], ps_rr[:])
    rr_len2 = work.tile([128, NE], F32, tag="rr_len2")
    nc.vector.tensor_tensor(rr_len2[:], rr2[:, 0:NE], rr2[:, NE:2 * NE],
                            op=Alu.add)
    nc.vector.tensor_tensor(rr_len2[:], rr_len2[:], rr2[:, 2 * NE:3 * NE],
                            op=Alu.add)
    rr_len = work.tile([128, NE], F32, tag="rr_len")
    nc.scalar.sqrt(rr_len[:], rr_len2[:])  # diagonal is exactly 0

    # ---------------- pT2_0: p_v0 in transposed-doubled layout ----------------
    # pT2_0[g, j*64+il] = p_v0[il, j, g] (u half, partitions 0..3)
    # pT2_0[64+g, ...] = p_v0[64+il, j, g] (d half, partitions 64..67)
    # The partition->free flattening DMAs are split by j-half so layer 0's
    # first chunks can start as soon as the low-j halves land.
    pT2_0 = big.tile([128, 8192], FP16, tag="pT2_0")
    for g in range(4):
        # c-major rr blocks are contiguous, so the TensorE transpose (which
        # silently no-ops for strided inputs on HW) reads them directly
        src = rr_sb[:, g * 128:(g + 1) * 128] if g < 3 else rr_len[:]
        pst = ps_sm.tile([128, 128], F32, tag="small")
        nc.tensor.transpose(pst[:], src, ident[:])  # pst[j, i] = p0[i, j, g]
        pstc = pipe2.tile([128, 128], FP16, tag="p0T")
        nc.vector.tensor_copy(pstc[:], pst[:])
        du = pT2_0[g:g + 1, :].rearrange("p (j i) -> p j i", i=64)
        dd = pT2_0[64 + g:65 + g, :].rearrange("p (j i) -> p j i", i=64)
        for jh in range(2):
            jsl = slice(jh * 64, (jh + 1) * 64)
            dma(du[:, jsl, :], pstc[jsl, 0:64])
            dma(dd[:, jsl, :], pstc[jsl, 64:128])

    ones128, maskNEG, UD, SELDIFF, diagmask, UDinv = secondary_consts()

    # ---------------- ra-path (feeds s_v0 / e_col / pm0; all consumed
    # by the s-stack, later than the p-stack) ----------------
    Wra = const.tile([4, 3 * NA], F32, tag="Wra")
    delta_rows(Wra, NA, 1.0)

    ps_ra = ps_sm.tile([128, 3 * NA], F32, tag="small")
    nc.tensor.matmul(ps_ra[:], rT4[0:3, :], Wra[0:3, :],
                     start=True, stop=False)
    nc.tensor.matmul(ps_ra[:], ones_row[:], arow_neg[:],
                     start=False, stop=True)
    ra_sb = work.tile([128, 3 * NA], F32, tag="ra_sb")
    nc.vector.tensor_copy(ra_sb[:], ps_ra[:])
    ra2 = work.tile([128, 3 * NA], F32, tag="ra2")
    nc.scalar.square(ra2[:], ps_ra[:])
    ra_len2 = work.tile([128, NA], F32, tag="ra_len2")
    nc.vector.reduce_sum(
        ra_len2[:], ra2[:].rearrange("p (j c) -> p j c", c=3),
        axis=mybir.AxisListType.X,
    )
    ra_len = work.tile([128, NA], F32, tag="ra_len")
    nc.scalar.sqrt(ra_len[:], ra_len2[:])
    # e_col[i] = sum_j exp(-|r_i - a_j|)
    e_col = const.tile([128, 1], F32, tag="e_col")
    eexp = work.tile([128, NA], F32, tag="eexp")
    nc.scalar.activation(eexp[:], ra_len[:], AF.Exp, scale=-1.0,
                         accum_out=e_col[:])

    def dbg_out(src_ap):
        o = work.tile([1, 1], F32, tag="out_sb")
        nc.scalar.mul(o[:], src_ap, 1.0)
        dma(out_d[:], o[:])

    # s_v0 [128, 128]: interleaved [ra_x, ra_y, ra_z, |ra|] per atom
    s_v0 = work.tile([128, 128], F32, tag="s_v0")
    v4 = s_v0[:].rearrange("p (j k) -> p j k", k=4)
    nc.vector.tensor_copy(v4[:, :, 0:3],
                          ra_sb[:].rearrange("p (j c) -> p j c", c=3))
    nc.vector.tensor_copy(v4[:, :, 3:4],
                          ra_len[:].rearrange("p (j k) -> p j k", k=1))

    if stage == "geom":
        dbg_out(e_col[0:1, :])
        release_all()
        return

    # ---------------- pm0: layer-0 p-means, built analytically --------------
    # mean over an i-half of rr[i, j, c] = r[j, c] - rbar_half[c] for the
    # three coordinate features (the i=j diagonal term is 0 on both sides);
    # only the |rr| feature needs a real mean (one matmul against rr_len,
    # transposed to rows via the PE). pm0 rows: 0-3 = u [x y z len],
    # 4-7 = d, row 8 = ones (carries the V0 bias as a 9th contraction row).
    # Rows 3..8 live at compute-unaddressable partitions, so they are filled
    # by SBUF-to-SBUF DMA, all off the critical path.
    pm0 = work.tile([128, 128], F32, tag="pm0")
    dma(pm0[8:9, :], ones_row[:])
    ps_rb = ps_sm.tile([3, 2], F32, tag="small")
    nc.tensor.matmul(ps_rb[:], r_sb[:], UDinv[:], start=True, stop=True)
    nc.vector.tensor_scalar_sub(pm0[0:3, :], rT4[0:3, :], ps_rb[0:3, 0:1])
    pm0d = pipe2.tile([3, 128], F32, tag="pm0d")
    nc.vector.tensor_scalar_sub(pm0d[:], rT4[0:3, :], ps_rb[0:3, 1:2])
    dma(pm0[4:7, :], pm0d[:])
    ps_L = ps_sm.tile([128, 2], F32, tag="small")
    nc.tensor.matmul(ps_L[:], rr_len[:], UDinv[:], start=True, stop=True)
    lcol = pipe2.tile([128, 2], F32, tag="lcol")
    nc.vector.tensor_copy(lcol[:], ps_L[:])
    ps_LT = ps_sm.tile([2, 128], F32, tag="small")
    nc.tensor.transpose(ps_LT[:], lcol[:], ident[:])
    lrow = pipe2.tile([2, 128], F32, tag="lrow")
    nc.vector.tensor_copy(lrow[:], ps_LT[:])
    dma(pm0[3:4, :], lrow[0:1, :])
    dma(pm0[7:8, :], lrow[1:2, :])

    # ---------------- p-layer weights (doubled to both partition halves).
    # NOTE: keep the two K<=64 quadrant matmuls per chunk -- a single K=128
    # block-diagonal fp16 matmul was measured ~2x SLOWER per chunk (fp16's
    # 1 cyc/col apparently holds only for K<=64; K=128 takes a second pass).
    Wp, Wpb, Kp = [], [], [4, 64, 64]
    for l, (wn, bn) in enumerate([("W0_w", "W0_b"), ("W1_w", "W1_b"),
                                  ("W2_w", "W2_b")]):
        K = Kp[l]
        wstage = pipe2.tile([64, 64], F32, tag="wstage")
        dma(wstage[0:K, :], ins[wn][:])
        wt = const.tile([128, 64], FP16, tag=f"wp{l}")
        nc.vector.tensor_copy(wt[0:K, :], wstage[0:K, :])
        nc.vector.tensor_copy(wt[64:64 + K, :], wstage[0:K, :])
        bc = const.tile([128, 1], F32, tag=f"wpb{l}")
        dma(bc[0:64, :], ins[bn][:].rearrange("(a k) -> a k", k=1))
        dma(bc[64:128, :], ins[bn][:].rearrange("(a k) -> a k", k=1))
        Wp.append(wt)
        Wpb.append(bc)

    # ---------------- LU selector matrices (built early on idle GpSimd) ----
    # SELS_k = SELS[:, k*128:(k+1)*128], SELS_k[q, p] = 1 iff
    # (q == k and p < 64) or (q == 64+k and p >= 64): the lhsT of a single
    # matmul that extracts pivot rows k (u) / 64+k (d) of A and broadcasts
    # each to its 64-partition half. SELS_k = UD * SELDIFF[:,k] + ident[:,64+k]
    SELS = const.tile([128, 64 * 128], F32, tag="SELS")
    for k in range(64):
        nc.gpsimd.tensor_scalar(
            SELS[:, k * 128:(k + 1) * 128], UD[:],
            SELDIFF[:, k:k + 1], ident[:, 64 + k:65 + k],
            op0=Alu.mult, op1=Alu.add)

    # ---------------- p-layers ----------------
    # t_{l+1} = tanh(W_l^T applied to p_v_l); p_v residuals kept distributed.
    t_tiles = []

    def p_layer(l, rhs_list, out_tag=None):
        """rhs_list: list of (tile, K) contributions summed pre-tanh."""
        out_t = big.tile([128, 8192], FP16, tag=out_tag or f"t{l + 1}")
        wt, bc = Wp[l], Wpb[l]
        for c in range(16):
            ps = ps_big.tile([128, 512], F32, tag="big512")
            sl = slice(c * 512, (c + 1) * 512)
            n = len(rhs_list)
            for idx, (src, K) in enumerate(rhs_list):
                # independent accumulation group per psum partition-range
                st, sp = idx == 0, idx == n - 1
                nc.tensor.matmul(ps[0:64, :], wt[0:K, :], src[0:K, sl],
                                 start=st, stop=sp, tile_position=(0, 0))
                # skip_group_check: the sim's zero-region tracking is
                # bank-global, but disjoint-partition groups are sound
                # (per-element has_written bits); verified numerically.
                nc.tensor.matmul(ps[64:128, :], wt[64:64 + K, :],
                                 src[64:64 + K, sl],
                                 start=st, stop=sp, tile_position=(64, 64),
                                 skip_group_check=True)
            nc.scalar.activation(out_t[:, sl], ps[:], AF.Tanh, bias=bc[:])
        t_tiles.append(out_t)
        return out_t

    t1 = p_layer(0, [(pT2_0, 4)])
    t2 = p_layer(1, [(t1, 64)])
    # residual input p_v2 = t1 + t2 pre-added on DVE (chunked so the adds
    # pipeline behind layer 1's tanh) -- halves layer 2's matmul count.
    t12 = big.tile([128, 8192], FP16, tag="t12")
    for c in range(8):
        sl = slice(c * 1024, (c + 1) * 1024)
        nc.vector.tensor_tensor(t12[:, sl], t1[:, sl], t2[:, sl],
                                op=Alu.add)
    # t3 reuses pT2_0's SBUF slot (pT2_0 is dead after layer 0)
    t3 = p_layer(2, [(t12, 64)], out_tag="pT2_0")

    # ---------------- p means (cumulative, scaled 1/64) ----------------
    # red_l[q, j] = sum_il t_l[q, j*64+il]; pmean chunks feed s-matmul lhsT.
    # Cumulative means are pre-summed on DVE (pm12, pm123) so each s-layer
    # needs only ONE pm chunk matmul. The reduces are EMITTED LAZILY (from
    # pm_fn inside each s-layer, with the pm chunk ordered last) so the
    # 8.6us DVE reduce of layer l runs while the PE chews layer l+1's
    # su/sd/sv chunks instead of blocking the layer's broadcast builds.
    pm_state = {}

    def make_pm_fn(t_src, tag, prev_key, out_key):
        """Two-phase lazy i-segment mean of a t-tile: fold_fn emits the
        half-width add (i 0:32 + 32:64, f32), finish_fn the 32-wide
        segmented reduce and the cumulative 1/64-scaled accumulate. The
        phases are emitted at different points inside the consuming s-layer
        so the 6us of DVE reduce work hides behind its chunk matmuls. The
        f32 fold tile is shared across layers (reduces are serial on DVE)."""
        state = {}

        def fold_fn():
            v = t_src[:].rearrange("p (j i) -> p j i", i=64)
            fold = work.tile([128, 4096], F32, tag="redfold")
            nc.vector.tensor_tensor(
                fold[:].rearrange("p (j i) -> p j i", i=32),
                v[:, :, 0:32], v[:, :, 32:64], op=Alu.add)
            state["fold"] = fold

        def finish_fn():
            red = work.tile([128, 128], F32, tag=tag)
            nc.vector.reduce_sum(
                red[:], state["fold"][:].rearrange("p (j i) -> p j i", i=32),
                axis=mybir.AxisListType.X,
            )
            pm = work.tile([128, 128], F32, tag="pm" + tag)
            if prev_key is None:
                nc.vector.tensor_scalar_mul(pm[:], red[:], 1.0 / 64.0)
            else:
                nc.vector.scalar_tensor_tensor(pm[:], red[:], 1.0 / 64.0,
                                               pm_state[prev_key][:],
                                               op0=Alu.mult, op1=Alu.add)
            pm_state[out_key] = pm
            return pm

        return fold_fn, finish_fn

    if stage == "p":
        f, fin = make_pm_fn(t1, "red1", None, "pm1")
        f()
        dbg_out(fin()[0:1, 0:1])
        release_all()
        return

    # ---------------- s-layers ----------------
    def s_means_bcast(s_v, width):
        """Column-mean of the u/d row-halves of s_v, broadcast to [128,128]
        lhsT tiles (bt[k, m] = mean[k]; the matmul replicates the mean-row
        to all 128 output rows). One K=128 matmul against UDinv yields both
        halves' means at once. Returns (su_tiles, sd_tiles) per chunk."""
        nch = width // 128
        su, sd = [], []
        for c in range(nch):
            sl = slice(c * 128, (c + 1) * 128)
            psm = ps_sm.tile([128, 2], F32, tag="small")
            nc.tensor.matmul(psm[:], s_v[:, sl], UDinv[:],
                             start=True, stop=True)
            for half, out_list in ((0, su), (1, sd)):
                bt = sbcast.tile([128, 128], F32, tag="sbcast")
                nc.vector.tensor_scalar_mul(bt[:], ones128[:],
                                            psm[:, half:half + 1])
                out_list.append(bt)
        return su, sd

    def s_transposes(s_v, width):
        out = []
        for c in range(width // 128):
            sl = slice(c * 128, (c + 1) * 128)
            pst = ps_sm.tile([128, 128], F32, tag="small")
            nc.tensor.transpose(pst[:], s_v[:, sl], ident[:])
            svt = svtp.tile([128, 128], F32, tag="svT")
            nc.vector.tensor_copy(svt[:], pst[:])
            out.append(svt)
        return out

    def s_layer(lname, wkey, bkey, chunks, bias_chunk=None):
        """chunks: (lhsT_ap, vw_row_start, K). All matmuls are plain f32
        (see module docstring). bias_chunk: index of a chunk whose LAST
        contraction row is the bias (lhsT row = ones); otherwise the bias
        is a trailing rank-1 ones x b matmul. Returns s_v [128,512] f32."""
        ps = ps_s.tile([128, 512], F32, tag="big512")
        n = len(chunks)
        for idx, (lhsT, row0, K) in enumerate(chunks):
            wv = wstream.tile([128, 512], F32, tag="vw")
            if idx == bias_chunk:
                dma(wv[0:K - 1, :], ins[wkey][row0:row0 + K - 1, :])
                dma(wv[K - 1:K, :],
                    ins[bkey][:].rearrange("(k a) -> k a", k=1))
            else:
                dma(wv[0:K, :], ins[wkey][row0:row0 + K, :])
            stop = (idx == n - 1) and bias_chunk is not None
            nc.tensor.matmul(ps[:], lhsT, wv[0:K, :],
                             start=(idx == 0), stop=stop)
        if bias_chunk is None:
            vb = wstream.tile([1, 512], F32, tag="vb")
            dma(vb[:], ins[bkey][:].rearrange("(k a) -> k a", k=1))
            nc.tensor.matmul(ps[:], ones_row[:], vb[:],
                             start=False, stop=True)
        # tanh in two half-width pieces: the next layer's mean matmuls and
        # transposes for the low columns start ~300ns earlier, shrinking the
        # serial layer-boundary latency.
        s_v = work.tile([128, 512], F32, tag=f"sv{lname}")
        nc.scalar.activation(s_v[:, 0:256], ps[:, 0:256], AF.Tanh)
        nc.scalar.activation(s_v[:, 256:512], ps[:, 256:512], AF.Tanh)
        return s_v

    # layer 0: fin = 392 = su(128) sd(128) pu(4) pd(4) sv(128); bias rides
    # as the 9th row of the pm chunk (pm0 row 8 = ones). pm chunk last.
    su0, sd0 = s_means_bcast(s_v0, 128)
    sv0T = s_transposes(s_v0, 128)
    s_v1 = s_layer(
        "1", "V0_w", "V0_b",
        [(su0[0][:], 0, 128), (sd0[0][:], 128, 128),
         (sv0T[0][:], 264, 128),
         (pm0[0:9, :], 256, 9)],
        bias_chunk=3,
    )

    # layers 1, 2, after: fin = 1664 = su(512) sd(512) pu(64) pd(64) sv(512)
    def big_s_layer(lname, wkey, bkey, s_v, pm_fns):
        fold_fn, finish_fn = pm_fns
        su, sd = s_means_bcast(s_v, 512)
        fold_fn()
        svT = s_transposes(s_v, 512)
        chunks = []
        for c in range(4):
            chunks.append((su[c][:], c * 128, 128))
        for c in range(4):
            chunks.append((sd[c][:], 512 + c * 128, 128))
        for c in range(4):
            chunks.append((svT[c][:], 1152 + c * 128, 128))
        # pu rows 1024:1088 and pd rows 1088:1152 are contiguous in Vw, and
        # pm holds pu-features at partitions 0:64, pd at 64:128 -- one
        # full-array K=128 chunk covers both. Emitted last: the DVE reduce
        # it depends on overlaps the 12 chunk matmuls above.
        pm = finish_fn()
        chunks.append((pm[:], 1024, 128))
        return s_layer(lname, wkey, bkey, chunks)

    if stage == "s1" and dbg_d is not None:
        sv1f = work.tile([128, 512], F32, tag="sv1f")
        nc.scalar.activation(sv1f[:], s_v1[:], AF.Identity)
        dma(dbg_d[:], sv1f[:])
        dbg_out(s_v1[0:1, 0:1])
        release_all()
        return

    s_v2 = big_s_layer("2", "V1_w", "V1_b", s_v1,
                       make_pm_fn(t1, "red1", None, "pm1"))
    s_v3 = big_s_layer("3", "V2_w", "V2_b", s_v2,
                       make_pm_fn(t2, "red2", "pm1", "pm12"))
    s_v4 = big_s_layer("4", "after_w", "after_b", s_v3,
                       make_pm_fn(t3, "red3", "pm12", "pm123"))

    if stage == "s":
        dbg_out(s_v4[0:1, 0:1])
        release_all()
        return

    # ---------------- heads ----------------
    sv4T = s_transposes(s_v4, 512)

    def head_half(wkey, bkey):
        ps = ps_sm.tile([64, 256], F32, tag="small")
        base = 0 if wkey == "vhu_w" else 64
        for c in range(4):
            wv = wstream.tile([128, 256], F32, tag="vhw")
            dma(wv[:], ins[wkey][c * 128:(c + 1) * 128, :])
            nc.tensor.matmul(ps[:], sv4T[c][:, base:base + 64],
                             wv[:], start=(c == 0), stop=False)
        vb = wstream.tile([1, 256], F32, tag="vhb")
        dma(vb[:], ins[bkey][:].rearrange("(k a) -> k a", k=1))
        nc.tensor.matmul(ps[:], ones_row[:, 0:64], vb[:],
                         start=False, stop=True)
        sh = work.tile([64, 256], F32, tag="sh" + wkey)
        nc.vector.tensor_copy(sh[:], ps[:])
        return sh

    shu = head_half("vhu_w", "vhu_b")
    shd = head_half("vhd_w", "vhd_b")

    def head_T(sh):
        out = []
        for c in range(2):
            pst = ps_sm.tile([128, 128], F32, tag="small")
            nc.tensor.transpose(pst[0:128, 0:64],
                                sh[:, c * 128:(c + 1) * 128],
                                ident[0:64, 0:64])
            ht = svtp.tile([128, 64], F32, tag="ht")
            nc.vector.tensor_copy(ht[:], pst[0:128, 0:64])
            out.append(ht)
        return out

    shuT = head_T(shu)
    shdT = head_T(shd)

    ps_A = ps_sm.tile([128, 64], F32, tag="small")
    for c in range(2):
        wv = wstream.tile([128, 64], F32, tag="ww")
        dma(wv[:], ins["wu_w"][c * 128:(c + 1) * 128, :])
        nc.tensor.matmul(ps_A[0:64, :], shuT[c][:], wv[:],
                         start=(c == 0), stop=False, tile_position=(0, 0))
    vbu = wstream.tile([1, 64], F32, tag="wb")
    dma(vbu[:], ins["wu_b"][:].rearrange("(k a) -> k a", k=1))
    nc.tensor.matmul(ps_A[0:64, :], ones_row[:, 0:64], vbu[:],
                     start=False, stop=True, tile_position=(0, 0))
    for c in range(2):
        wv = wstream.tile([128, 64], F32, tag="ww")
        dma(wv[:], ins["wd_w"][c * 128:(c + 1) * 128, :])
        nc.tensor.matmul(ps_A[64:128, :], shdT[c][:], wv[:],
                         start=(c == 0), stop=False, tile_position=(0, 64))
    vbd = wstream.tile([1, 64], F32, tag="wb")
    dma(vbd[:], ins["wd_b"][:].rearrange("(k a) -> k a", k=1))
    nc.tensor.matmul(ps_A[64:128, :], ones_row[:, 0:64], vbd[:],
                     start=False, stop=True, tile_position=(0, 64))

    # orb = s_w * (sum_j exp(-|ra|)) row-scale; stacked [A_u; A_d]
    A_sb = work.tile([128, 64], F32, tag="A_sb")
    nc.vector.tensor_scalar_mul(A_sb[:], ps_A[:], e_col[:])
    # dummy Ln: pulls the ACT table load for the logdet epilogue off the
    # end of the serial LU (the ACT engine is idle for the whole LU)
    lnwarm = work.tile([1, 1], F32, tag="lnwarm")
    nc.scalar.activation(lnwarm[:], ones_row[:, 0:1], AF.Ln)

    if stage == "heads":
        dbg_out(A_sb[0:1, 0:1])
        release_all()
        return

    # ---------------- stacked unpivoted LU ----------------
    # Per step k: one matmul (lhsT = SELS_k) extracts pivot rows k / 64+k of
    # A and broadcasts each across its partition half into PSUM; then 3 DVE
    # ops: R = 1/UB[:,k] (the DVE has no divide ALU op -- the ISA check
    # rejects it), m2 = A[:,k] * R * maskNEG[:,k] (zero for rows <= k, so
    # finished U rows and the diagonal are never touched) and the fused
    # rank-1 update A = UB * m2 + A over the full 64-col width (dead columns
    # only ever receive ~eps noise; the multiplier column k is eliminated to
    # ~0, which is dead anyway). Step 63 does nothing: row 63's diagonal was
    # finalized by step 62. The U diagonal is gathered from A at the end.
    n_lu = {"lu16": 16, "lu64": 63}.get(stage, 63)
    for k in range(n_lu):
        # only the active columns k..63 are extracted and updated; stale
        # sub-diagonal columns are dead (every live (row k', col >= k')
        # entry has received all updates < k' because step t covers cols
        # t+1..63 in full).
        w = 64 - k
        ub = ps_sm.tile([128, w], F32, tag="small")
        nc.tensor.matmul(ub[:], SELS[:, k * 128:(k + 1) * 128],
                         A_sb[:, k:64], start=True, stop=True)
        rc = lu.tile([128, 1], F32, tag="rc")
        nc.vector.reciprocal(rc[:], ub[:, 0:1])
        m2 = lu.tile([128, 1], F32, tag="m2")
        nc.vector.tensor_scalar(m2[:], A_sb[:, k:k + 1], rc[:],
                                maskNEG[:, k:k + 1],
                                op0=Alu.mult, op1=Alu.mult)
        nc.vector.scalar_tensor_tensor(A_sb[:, k + 1:64], ub[:, 1:w],
                                       m2[:], A_sb[:, k + 1:64],
                                       op0=Alu.mult, op1=Alu.add)

    if stage in ("lu16", "lu64"):
        dbg_out(A_sb[0:1, 0:1])
        release_all()
        return

    # logdet = allreduce_p( ln|A[p, p%64]| )  (each partition holds one
    # diagonal element of its half's U factor)
    scr = work.tile([128, 64], F32, tag="scr")
    diag_col = work.tile([128, 1], F32, tag="diag_col")
    nc.vector.tensor_tensor(scr[:], A_sb[:], diagmask[:], op=Alu.mult)
    nc.vector.reduce_sum(diag_col[:], scr[:], axis=mybir.AxisListType.X)
    # ln|d| = ln(d^2)/2 -- avoids the Abs table load (only Ln loads)
    sq_col = work.tile([128, 1], F32, tag="sq_col")
    nc.vector.tensor_tensor(sq_col[:], diag_col[:], diag_col[:],
                            op=Alu.mult)
    ln_col = work.tile([128, 1], F32, tag="ln_col")
    nc.scalar.activation(ln_col[:], sq_col[:], AF.Ln)
    # cross-partition sum: transpose the column to a row, reduce along free
    ps_out = ps_sm.tile([1, 128], F32, tag="small")
    nc.tensor.transpose(ps_out[:], ln_col[:], ident[:])
    s_row = work.tile([1, 128], F32, tag="s_row")
    nc.vector.tensor_scalar_mul(s_row[:], ps_out[:], 0.5)
    out_sb = work.tile([1, 1], F32, tag="out_sb")
    nc.vector.reduce_sum(out_sb[:], s_row[:], axis=mybir.AxisListType.X)
    dma(out_d[:], out_sb[:])

    release_all()


_NC_CACHE = {}


def build_nc():
    if "nc" in _NC_CACHE:
        return _NC_CACHE["nc"]
    import os
    nc = bacc.Bacc("TRN2", target_bir_lowering=False, debug=False)
    ins = {}
    for name, shape in INPUT_SPECS:
        ins[name] = nc.dram_tensor(name, list(shape), F32,
                                   kind="ExternalInput").ap()
    out_d = nc.dram_tensor("out", [1, 1], F32, kind="ExternalOutput").ap()
    dbg_d = None
    if os.environ.get("KSTAGE", "full") in ("s1", "pm0"):
        dbg_d = nc.dram_tensor("dbgout", [128, 512], F32,
                               kind="ExternalOutput").ap()
    with tile.TileContext(nc) as tc:
        _program(tc, nc, ins, out_d, dbg_d)
    nc.compile()
    _NC_CACHE["nc"] = nc
    return nc


def kernel(**inputs) -> np.ndarray:
    from concourse.bass_utils import run_bass_kernel_spmd

    nc = build_nc()
    in_map = {name: np.ascontiguousarray(np.asarray(inputs[name],
                                                    dtype=np.float32))
              for name, _ in INPUT_SPECS}
    in_maps = [in_map for _ in range(8)]
    res = run_bass_kernel_spmd(nc, in_maps, core_ids=list(range(8)))
    out = res.results[0]["out"]
    return np.float32(out.reshape(())[()])
